# revision 39
# baseline (speedup 1.0000x reference)
"""DKVMN scatter_memory kernel for 8 Trainium2 NeuronCores.

Math: the reference scan only ever uses the (B, M, Dv) memory through
read @ Wf_r, so the whole recurrence collapses to a 32-dim linear
cumulative sum:

  S  = softmax(Eq @ Wa + ba)            (100 x 32)  per-vocab att rows
  cq = Eq @ Wf[:64] + bf                (100,)
  cv = Ev @ Wf[64:]                     (100,)
  w  = (2q + a) % 100
  pred[t,b] = cq[q[t,b]] + sum_{s<t} cv[w[s,b]] * <S[q[t,b]], S[q[s,b]]>

Per core (batch-sharded, Bs=128): the host precomputes a 120-row fp8
index encoding per token (pure index preprocessing; 0/1 exact in fp8):
rows 0:100 one-hot(q), rows 100:110 one-hot(w//10), rows 110:120
one-hot(w%10).  One 53-col matmul per batch element against a packed
table [S | cq | cvtab | I10] gathers the S-row, cq, and the digit
factors of cv[w] (cv[w] = sum_j 1{w1=j} * cv[10j+w0], with cvtab
reshaped from cv on device).  The cumsum over t is a strict-upper-
triangular matmul.  Layout: t on partitions, (b, m) on free dim.
"""
import functools
import numpy as np
import ml_dtypes

import concourse.bass as bass
import concourse.bacc as bacc
import concourse.mybir as mybir
from concourse import tile
from concourse.bass_utils import run_bass_kernel_spmd

T, B, M, DQ, DV, VOCAB = 128, 1024, 32, 64, 64, 100
NCORES = 8
BS = B // NCORES  # 128
N = T * BS        # tokens per core = 16384
R = 120           # one-hot rows: 100 q + 10 w-hi + 10 w-lo
GB = 16           # b per group; per-b psum stride 64 (53 used), 2 banks
GROUPS = [(g * GB, GB) for g in range(8)]
GCOL = [b0 * T for b0, _ in GROUPS]
F32 = mybir.dt.float32
F16 = mybir.dt.float16
FP8 = mybir.dt.float8e4
AX = mybir.AxisListType
OP = mybir.AluOpType

# packed-parameter column layout (f16 [128, PC]); EqT/EvT pre-transposed
_EQT, _EVT, _WA, _WFQ, _WFR = 0, 100, 200, 232, 233
_US, _ONE, _BA, _BF, _I10 = 234, 362, 462, 494, 495
PC = 505

# one-hot chunk schedule: (queue, [group indices])
CHUNKS = [
    ("sync", [0, 1]),
    ("scalar", [2, 3]),
    ("sync", [4, 5]),
    ("scalar", [6, 7]),
]


def _build():
    nc = bacc.Bacc("TRN2", num_devices=NCORES, debug=False, target_bir_lowering=False)
    d = {}
    d["pack"] = nc.dram_tensor("pack", [128, PC], F16, kind="ExternalInput").ap()
    d["ohall"] = nc.dram_tensor("ohall", [R, N], FP8, kind="ExternalInput").ap()
    preds = nc.dram_tensor("preds", [T, BS], F32, kind="ExternalOutput").ap()

    with tile.TileContext(nc) as tc:
        with (
            tc.tile_pool(name="sb", bufs=1) as sb,
            tc.tile_pool(name="wk", bufs=3) as wk,
            tc.tile_pool(name="ps", bufs=2, space="PSUM") as ps,
        ):
            P = sb.tile([128, PC], F16)
            nc.scalar.dma_start(P[:], d["pack"][:])
            gtile = [None] * len(GROUPS)
            goff = [0] * len(GROUPS)

            def load_chunk(ci):
                qname, gids = CHUNKS[ci]
                eng = getattr(nc, qname)
                c0 = GCOL[gids[0]]
                g_end = gids[-1]
                c1 = GCOL[g_end] + GROUPS[g_end][1] * T
                t_ = sb.tile([R, c1 - c0], FP8, name=f"oh_chunk_{ci}")
                eng.dma_start(t_[:], d["ohall"][:, c0:c1])
                for g in gids:
                    gtile[g] = t_
                    goff[g] = GCOL[g] - c0

            for ci in range(len(CHUNKS)):
                load_chunk(ci)

            us_t = P[:, _US:_US + 128]

            # ---- parameter tables (EqT / EvT arrive pre-transposed) ----
            # mcat = [S | cq | cvtab | I10]  (120 x 53) fp16
            mcat = sb.tile([R, 53], F16)
            nc.vector.memset(mcat[:], 0.0)

            # EvT columns arrive permuted (perm(k) = 10(k%10) + k//10), so the
            # cv row comes out as cv_row[0, 10i+j] = cv[10j+i]; a [1,100] ->
            # [10,10] DMA spray then yields cvtab[i, j] = cv[10j+i].
            p_cvr = ps.tile([1, VOCAB], F32, tag="pA")
            nc.tensor.matmul(p_cvr[:], P[0:DV, _WFR:_WFR + 1],
                             P[0:DV, _EVT:_EVT + VOCAB], start=True, stop=True)
            cv_row = sb.tile([1, VOCAB], F16)
            nc.scalar.copy(cv_row[:], p_cvr[:])
            nc.sync.dma_start(mcat[110:120, 33:43], cv_row[0:1, 0:VOCAB])
            # I10 at rows 100:110 (DMA: engines can't address partition 100+)
            nc.scalar.dma_start(mcat[100:110, 43:53], d["pack"][100:110, _I10:_I10 + 10])

            eqT_t = P[0:DQ, _EQT:_EQT + VOCAB]
            p_s = ps.tile([VOCAB, M], F32, tag="pA")
            nc.tensor.matmul(p_s[:], eqT_t, P[0:DQ, _WA:_WA + M], start=True, stop=False)
            nc.tensor.matmul(p_s[:], P[0:1, _ONE:_ONE + VOCAB], P[0:1, _BA:_BA + M],
                             start=False, stop=True)
            mx_t = sb.tile([VOCAB, 1], F32)
            sm_t = sb.tile([VOCAB, 1], F32)
            se_t = sb.tile([VOCAB, M], F32)
            nc.vector.tensor_reduce(mx_t[:], p_s[:], AX.X, OP.max)
            nc.vector.tensor_scalar_mul(mx_t[:], mx_t[:], -1.0)
            nc.scalar.activation(se_t[:], p_s[:],
                                 mybir.ActivationFunctionType.Exp,
                                 bias=mx_t[:], scale=1.0)
            nc.vector.tensor_reduce(sm_t[:], se_t[:], AX.X, OP.add)
            nc.vector.reciprocal(sm_t[:], sm_t[:])
            nc.vector.tensor_scalar(out=mcat[0:VOCAB, 0:M], in0=se_t[:], scalar1=sm_t[:],
                                    scalar2=None, op0=OP.mult)
            p_cq = ps.tile([VOCAB, 1], F32, tag="pP")
            nc.tensor.matmul(p_cq[:], eqT_t, P[0:DQ, _WFQ:_WFQ + 1], start=True, stop=False)
            nc.tensor.matmul(p_cq[:], P[0:1, _ONE:_ONE + VOCAB], P[0:1, _BF:_BF + 1],
                             start=False, stop=True)
            nc.scalar.copy(mcat[0:VOCAB, M:M + 1], p_cq[:])

            # ---- main pipeline ----
            out_sb = sb.tile([128, BS], F32)
            out16 = sb.tile([128, BS], F16)
            c_sb = sb.tile([128, BS], F32)

            for pi in range(4):
                gis = [2 * pi, 2 * pi + 1]
                icvt = wk.tile([128, 640], F16, tag="icvt_sb")
                cvp = wk.tile([128, 320], F16, tag="cvp_sb")
                cvw_g = wk.tile([128, 2 * GB], F16, tag="cvw_sb")
                ap_p = wk.tile([128, 1024], F16, tag="ap_sb")
                pAs = []
                for half, gi in enumerate(gis):
                    b0, gb = GROUPS[gi]
                    oh_g = gtile[gi]
                    off = goff[gi]
                    pA = ps.tile([128, 1024], F32, tag="pA", name=f"pA_{half}")
                    for k in range(gb):
                        nc.tensor.matmul(pA[:, k * 64:k * 64 + 53],
                                         oh_g[:, off + k * T:off + (k + 1) * T],
                                         mcat[:], start=True, stop=True)
                    pA3 = pA[:].rearrange("p (k c) -> p k c", c=64)
                    nc.scalar.copy(c_sb[:, b0:b0 + gb], pA3[:, :, M:M + 1])
                    nc.scalar.copy(
                        icvt[:, half * 320:(half + 1) * 320].rearrange(
                            "p (k c) -> p k c", c=20),
                        pA3[:, :, 33:53])
                    pAs.append(pA)
                # cv[w] = sum_j cvt[j] * ind[j]  (both groups in one op)
                ic3 = icvt[:].rearrange("p (k c) -> p k c", c=20)
                nc.vector.tensor_tensor(
                    cvp[:].rearrange("p (k c) -> p k c", c=10),
                    ic3[:, :, 0:10], ic3[:, :, 10:20], OP.mult)
                with nc.allow_low_precision(reason="10-term f16 dot of one-hot"):
                    nc.vector.tensor_reduce(
                        cvw_g[:],
                        cvp[:].rearrange("p (k c) -> p k c", c=10),
                        AX.X, OP.add)
                for half, gi in enumerate(gis):
                    b0, gb = GROUPS[gi]
                    pA3 = pAs[half][:].rearrange("p (k c) -> p k c", c=64)
                    pP = ps.tile([128, 512], F32, tag="pP", name=f"pP_{half}")
                    a_g = wk.tile([128, 512], F16, tag="a_sb", name=f"a_{half}")
                    v_g = wk.tile([128, 512], F16, tag="v_sb", name=f"v_{half}")
                    nc.scalar.copy(a_g[:].rearrange("p (k c) -> p k c", c=M),
                                   pA3[:, :, 0:M])
                    # v = A * cv[w] (broadcast cvw along m)
                    a3 = a_g[:].rearrange("p (k c) -> p k c", c=M)
                    cvb = cvw_g[:, half * GB:(half + 1) * GB].rearrange(
                        "p (k c) -> p k c", c=1)
                    a3b, cvb = bass.broadcast_tensor_aps(a3, cvb)
                    nc.vector.tensor_tensor(
                        v_g[:].rearrange("p (k c) -> p k c", c=M),
                        a3b, cvb, OP.mult)
                    # exclusive cumsum over t (strict upper as lhsT)
                    nc.tensor.matmul(pP[:], us_t, v_g[:], start=True, stop=True)
                    # pred contribution terms: A * C
                    nc.vector.tensor_tensor(
                        ap_p[:, half * 512:(half + 1) * 512], a_g[:], pP[:],
                        OP.mult)
                b0 = GROUPS[gis[0]][0]
                with nc.allow_low_precision(reason="32-term f16 dot, tol 2e-2"):
                    nc.vector.tensor_reduce(
                        out16[:, b0:b0 + 2 * GB],
                        ap_p[:].rearrange("p (b m) -> p b m", m=M),
                        AX.X, OP.add)

            nc.vector.tensor_add(out_sb[:], out16[:], c_sb[:])
            nc.sync.dma_start(preds[:], out_sb[:])

    nc.compile()
    return nc


@functools.lru_cache(maxsize=1)
def _get_nc():
    return _build()


def _in_maps(questions, answers, Eq, Ev, Wa, ba, Wf, bf):
    questions = np.asarray(questions)
    answers = np.asarray(answers)
    w = (questions.astype(np.int64) * 2 + answers.astype(np.int64)) % VOCAB
    pack = np.zeros((128, PC), np.float16)
    pack[0:DQ, _EQT:_EQT + VOCAB] = np.asarray(Eq, np.float32).T
    # EvT columns permuted so the derived cv row is emitted in (i-major) order
    perm = np.array([10 * (k % 10) + k // 10 for k in range(VOCAB)])
    pack[0:DV, _EVT:_EVT + VOCAB] = np.asarray(Ev, np.float32)[perm].T
    pack[0:DQ, _WA:_WA + M] = np.asarray(Wa, np.float32)
    wf = np.asarray(Wf, np.float32).reshape(DQ + DV)
    pack[0:DQ, _WFQ] = wf[0:DQ]
    pack[0:DV, _WFR] = wf[DQ:DQ + DV]
    pack[:, _US:_US + 128] = np.triu(np.ones((128, 128), np.float16), k=1)
    pack[0, _ONE:_ONE + VOCAB] = 1.0
    pack[0, _BA:_BA + M] = np.asarray(ba, np.float32).reshape(M)
    pack[0, _BF] = np.asarray(bf, np.float32).reshape(())
    pack[100:110, _I10:_I10 + 10] = np.eye(10, dtype=np.float16)
    in_maps = []
    for c in range(NCORES):
        sl = slice(c * BS, (c + 1) * BS)
        qf = np.ascontiguousarray(questions[:, sl].T).ravel()
        wfl = np.ascontiguousarray(w[:, sl].T).ravel()
        oh = np.zeros((R, N), dtype=ml_dtypes.float8_e4m3)
        ar = np.arange(N)
        oh[qf, ar] = 1.0
        oh[100 + wfl // 10, ar] = 1.0
        oh[110 + wfl % 10, ar] = 1.0
        in_maps.append({"pack": pack, "ohall": oh})
    return in_maps


def kernel(questions, answers, Eq, Ev, Wa, ba, Wf, bf):
    nc = _get_nc()
    in_maps = _in_maps(questions, answers, Eq, Ev, Wa, ba, Wf, bf)
    res = run_bass_kernel_spmd(nc, in_maps, list(range(NCORES)))
    preds = np.concatenate([res.results[c]["preds"] for c in range(NCORES)], axis=1)
    return preds.astype(np.float32)


# revision 40
# speedup vs baseline: 1.1435x; 1.1435x over previous
"""DKVMN scatter_memory kernel for 8 Trainium2 NeuronCores.

Math: the reference scan only ever uses the (B, M, Dv) memory through
read @ Wf_r, so the whole recurrence collapses to a 32-dim linear
cumulative sum:

  S  = softmax(Eq @ Wa + ba)            (100 x 32)  per-vocab att rows
  cq = Eq @ Wf[:64] + bf                (100,)
  cv = Ev @ Wf[64:]                     (100,)
  w  = (2q + a) % 100
  pred[t,b] = cq[q[t,b]] + sum_{s<t} cv[w[s,b]] * <S[q[t,b]], S[q[s,b]]>

Per core (batch-sharded, Bs=128): the host precomputes a 120-row fp8
index encoding per token (pure index preprocessing; 0/1 exact in fp8):
rows 0:100 one-hot(q), rows 100:110 one-hot(w//10), rows 110:120
one-hot(w%10).  One 53-col matmul per batch element against a packed
table [S | cq | cvtab | I10] gathers the S-row, cq, and the digit
factors of cv[w] (cv[w] = sum_j 1{w1=j} * cv[10j+w0], with cvtab
reshaped from cv on device).  The cumsum over t is a strict-upper-
triangular matmul.  Layout: t on partitions, (b, m) on free dim.
"""
import functools
import numpy as np
import ml_dtypes

import concourse.bass as bass
import concourse.bacc as bacc
import concourse.mybir as mybir
from concourse import tile
from concourse.bass_utils import run_bass_kernel_spmd

T, B, M, DQ, DV, VOCAB = 128, 1024, 32, 64, 64, 100
NCORES = 8
BS = B // NCORES  # 128
N = T * BS        # tokens per core = 16384
R = 120           # one-hot rows: 100 q + 10 w-hi + 10 w-lo
GB = 16           # b per group; per-b psum stride 64 (53 used), 2 banks
GROUPS = [(g * GB, GB) for g in range(8)]
GCOL = [b0 * T for b0, _ in GROUPS]
F32 = mybir.dt.float32
F16 = mybir.dt.float16
FP8 = mybir.dt.float8e4
AX = mybir.AxisListType
OP = mybir.AluOpType

# packed-parameter column layout (f16 [128, PC])
_EQ, _EV, _WA, _WFQ, _WFR = 0, 64, 128, 160, 161
_ID, _US, _ONE, _BA, _BF, _I10 = 162, 262, 390, 490, 522, 523
PC = 533

# one-hot chunk schedule: (queue, [group indices])
CHUNKS = [
    ("sync", [0, 1]),
    ("scalar", [2, 3]),
    ("sync", [4, 5]),
    ("scalar", [6, 7]),
]


def _build():
    nc = bacc.Bacc("TRN2", num_devices=NCORES, debug=False, target_bir_lowering=False)
    d = {}
    d["pack"] = nc.dram_tensor("pack", [128, PC], F16, kind="ExternalInput").ap()
    d["ohall"] = nc.dram_tensor("ohall", [R, N], FP8, kind="ExternalInput").ap()
    preds = nc.dram_tensor("preds", [T, BS], F32, kind="ExternalOutput").ap()

    with tile.TileContext(nc) as tc:
        with (
            tc.tile_pool(name="sb", bufs=1) as sb,
            tc.tile_pool(name="wk", bufs=3) as wk,
            tc.tile_pool(name="ps", bufs=2, space="PSUM") as ps,
        ):
            P = sb.tile([128, PC], F16)
            nc.scalar.dma_start(P[:], d["pack"][:])
            gtile = [None] * len(GROUPS)
            goff = [0] * len(GROUPS)
            for ci, (qname, gids) in enumerate(CHUNKS):
                eng = getattr(nc, qname)
                c0 = GCOL[gids[0]]
                g_end = gids[-1]
                c1 = GCOL[g_end] + GROUPS[g_end][1] * T
                t_ = sb.tile([R, c1 - c0], FP8, name=f"oh_chunk_{ci}")
                eng.dma_start(t_[:], d["ohall"][:, c0:c1])
                for g in gids:
                    gtile[g] = t_
                    goff[g] = GCOL[g] - c0

            us_t = P[:, _US:_US + 128]

            # ---- parameter tables ----
            # cv row first: it feeds the mcat spray DMA (longest dep chain)
            p_evT = ps.tile([DV, VOCAB], F16, tag="pP")
            evT_t = sb.tile([DV, VOCAB], F16)
            nc.tensor.transpose(p_evT[:], P[0:VOCAB, _EV:_EV + DV], P[0:VOCAB, _ID:_ID + VOCAB])
            nc.scalar.copy(evT_t[:], p_evT[:])

            # mcat = [S | cq | cvtab | I10]  (120 x 53) fp16
            mcat = sb.tile([R, 53], F16)
            nc.vector.memset(mcat[:], 0.0)

            # Ev arrives row-permuted (perm(k) = 10(k%10) + k//10), so the cv
            # row comes out as cv_row[0, 10i+j] = cv[10j+i]; a plain [1,100]
            # -> [10,10] DMA spray then yields cvtab[i, j] = cv[10j+i].
            p_cvr = ps.tile([1, VOCAB], F32, tag="pA")
            nc.tensor.matmul(p_cvr[:], P[0:DV, _WFR:_WFR + 1], evT_t[:],
                             start=True, stop=True)
            cv_row = sb.tile([1, VOCAB], F16)
            nc.scalar.copy(cv_row[:], p_cvr[:])
            nc.gpsimd.dma_start(mcat[110:120, 33:43], cv_row[0:1, 0:VOCAB])
            # I10 at rows 100:110 (DMA: engines can't address partition 100+)
            nc.scalar.dma_start(mcat[100:110, 43:53], d["pack"][100:110, _I10:_I10 + 10])

            p_eqT = ps.tile([DQ, VOCAB], F16, tag="pA")
            eqT_t = sb.tile([DQ, VOCAB], F16)
            nc.tensor.transpose(p_eqT[:], P[0:VOCAB, _EQ:_EQ + DQ], P[0:VOCAB, _ID:_ID + VOCAB])
            nc.scalar.copy(eqT_t[:], p_eqT[:])
            p_s = ps.tile([VOCAB, M], F32, tag="pA")
            nc.tensor.matmul(p_s[:], eqT_t[:], P[0:DQ, _WA:_WA + M], start=True, stop=False)
            nc.tensor.matmul(p_s[:], P[0:1, _ONE:_ONE + VOCAB], P[0:1, _BA:_BA + M],
                             start=False, stop=True)
            mx_t = sb.tile([VOCAB, 1], F32)
            sm_t = sb.tile([VOCAB, 1], F32)
            se_t = sb.tile([VOCAB, M], F32)
            nc.vector.tensor_reduce(mx_t[:], p_s[:], AX.X, OP.max)
            nc.vector.tensor_scalar_mul(mx_t[:], mx_t[:], -1.0)
            nc.scalar.activation(se_t[:], p_s[:],
                                 mybir.ActivationFunctionType.Exp,
                                 bias=mx_t[:], scale=1.0)
            nc.vector.tensor_reduce(sm_t[:], se_t[:], AX.X, OP.add)
            nc.vector.reciprocal(sm_t[:], sm_t[:])
            nc.vector.tensor_scalar(out=mcat[0:VOCAB, 0:M], in0=se_t[:], scalar1=sm_t[:],
                                    scalar2=None, op0=OP.mult)
            p_cq = ps.tile([VOCAB, 1], F32, tag="pP")
            nc.tensor.matmul(p_cq[:], eqT_t[:], P[0:DQ, _WFQ:_WFQ + 1], start=True, stop=False)
            nc.tensor.matmul(p_cq[:], P[0:1, _ONE:_ONE + VOCAB], P[0:1, _BF:_BF + 1],
                             start=False, stop=True)
            nc.scalar.copy(mcat[0:VOCAB, M:M + 1], p_cq[:])

            # ---- main pipeline ----
            out_sb = sb.tile([128, BS], F32)
            out16 = sb.tile([128, BS], F16)
            c_sb = sb.tile([128, BS], F32)

            for pi in range(4):
                gis = [2 * pi, 2 * pi + 1]
                icvt = wk.tile([128, 640], F16, tag="icvt_sb")
                cvp = wk.tile([128, 320], F16, tag="cvp_sb")
                cvw_g = wk.tile([128, 2 * GB], F16, tag="cvw_sb")
                ap_p = wk.tile([128, 1024], F16, tag="ap_sb")
                pAs = []
                for half, gi in enumerate(gis):
                    b0, gb = GROUPS[gi]
                    oh_g = gtile[gi]
                    off = goff[gi]
                    pA = ps.tile([128, 1024], F32, tag="pA", name=f"pA_{half}")
                    for k in range(gb):
                        nc.tensor.matmul(pA[:, k * 64:k * 64 + 53],
                                         oh_g[:, off + k * T:off + (k + 1) * T],
                                         mcat[:], start=True, stop=True)
                    pA3 = pA[:].rearrange("p (k c) -> p k c", c=64)
                    nc.scalar.copy(c_sb[:, b0:b0 + gb], pA3[:, :, M:M + 1])
                    nc.scalar.copy(
                        icvt[:, half * 320:(half + 1) * 320].rearrange(
                            "p (k c) -> p k c", c=20),
                        pA3[:, :, 33:53])
                    pAs.append(pA)
                # cv[w] = sum_j cvt[j] * ind[j]  (both groups in one op)
                ic3 = icvt[:].rearrange("p (k c) -> p k c", c=20)
                nc.vector.tensor_tensor(
                    cvp[:].rearrange("p (k c) -> p k c", c=10),
                    ic3[:, :, 0:10], ic3[:, :, 10:20], OP.mult)
                with nc.allow_low_precision(reason="10-term f16 dot of one-hot"):
                    nc.vector.tensor_reduce(
                        cvw_g[:],
                        cvp[:].rearrange("p (k c) -> p k c", c=10),
                        AX.X, OP.add)
                for half, gi in enumerate(gis):
                    b0, gb = GROUPS[gi]
                    pA3 = pAs[half][:].rearrange("p (k c) -> p k c", c=64)
                    pP = ps.tile([128, 512], F32, tag="pP", name=f"pP_{half}")
                    a_g = wk.tile([128, 512], F16, tag="a_sb", name=f"a_{half}")
                    v_g = wk.tile([128, 512], F16, tag="v_sb", name=f"v_{half}")
                    nc.scalar.copy(a_g[:].rearrange("p (k c) -> p k c", c=M),
                                   pA3[:, :, 0:M])
                    # v = A * cv[w] (broadcast cvw along m)
                    a3 = a_g[:].rearrange("p (k c) -> p k c", c=M)
                    cvb = cvw_g[:, half * GB:(half + 1) * GB].rearrange(
                        "p (k c) -> p k c", c=1)
                    a3b, cvb = bass.broadcast_tensor_aps(a3, cvb)
                    nc.vector.tensor_tensor(
                        v_g[:].rearrange("p (k c) -> p k c", c=M),
                        a3b, cvb, OP.mult)
                    # exclusive cumsum over t (strict upper as lhsT)
                    nc.tensor.matmul(pP[:], us_t, v_g[:], start=True, stop=True)
                    # pred contribution terms: A * C
                    nc.vector.tensor_tensor(
                        ap_p[:, half * 512:(half + 1) * 512], a_g[:], pP[:],
                        OP.mult)
                b0 = GROUPS[gis[0]][0]
                with nc.allow_low_precision(reason="32-term f16 dot, tol 2e-2"):
                    nc.vector.tensor_reduce(
                        out16[:, b0:b0 + 2 * GB],
                        ap_p[:].rearrange("p (b m) -> p b m", m=M),
                        AX.X, OP.add)

            nc.vector.tensor_add(out_sb[:], out16[:], c_sb[:])
            nc.sync.dma_start(preds[:], out_sb[:])

    nc.compile()
    return nc


@functools.lru_cache(maxsize=1)
def _get_nc():
    return _build()


def _in_maps(questions, answers, Eq, Ev, Wa, ba, Wf, bf):
    questions = np.asarray(questions)
    answers = np.asarray(answers)
    w = (questions.astype(np.int64) * 2 + answers.astype(np.int64)) % VOCAB
    pack = np.zeros((128, PC), np.float16)
    pack[0:VOCAB, _EQ:_EQ + DQ] = np.asarray(Eq, np.float32)
    # Ev rows permuted so the derived cv row is emitted in (i-major) order
    perm = np.array([10 * (k % 10) + k // 10 for k in range(VOCAB)])
    pack[0:VOCAB, _EV:_EV + DV] = np.asarray(Ev, np.float32)[perm]
    pack[0:DQ, _WA:_WA + M] = np.asarray(Wa, np.float32)
    wf = np.asarray(Wf, np.float32).reshape(DQ + DV)
    pack[0:DQ, _WFQ] = wf[0:DQ]
    pack[0:DV, _WFR] = wf[DQ:DQ + DV]
    pack[0:VOCAB, _ID:_ID + VOCAB] = np.eye(VOCAB, dtype=np.float16)
    pack[:, _US:_US + 128] = np.triu(np.ones((128, 128), np.float16), k=1)
    pack[0, _ONE:_ONE + VOCAB] = 1.0
    pack[0, _BA:_BA + M] = np.asarray(ba, np.float32).reshape(M)
    pack[0, _BF] = np.asarray(bf, np.float32).reshape(())
    pack[100:110, _I10:_I10 + 10] = np.eye(10, dtype=np.float16)
    in_maps = []
    for c in range(NCORES):
        sl = slice(c * BS, (c + 1) * BS)
        qf = np.ascontiguousarray(questions[:, sl].T).ravel()
        wfl = np.ascontiguousarray(w[:, sl].T).ravel()
        oh = np.zeros((R, N), dtype=ml_dtypes.float8_e4m3)
        ar = np.arange(N)
        oh[qf, ar] = 1.0
        oh[100 + wfl // 10, ar] = 1.0
        oh[110 + wfl % 10, ar] = 1.0
        in_maps.append({"pack": pack, "ohall": oh})
    return in_maps


def kernel(questions, answers, Eq, Ev, Wa, ba, Wf, bf):
    nc = _get_nc()
    in_maps = _in_maps(questions, answers, Eq, Ev, Wa, ba, Wf, bf)
    res = run_bass_kernel_spmd(nc, in_maps, list(range(NCORES)))
    preds = np.concatenate([res.results[c]["preds"] for c in range(NCORES)], axis=1)
    return preds.astype(np.float32)
